# revision 12
# baseline (speedup 1.0000x reference)
"""Trainium2 Bass kernel for nn_BaselineGRU: 2-layer GRU (B=16,T=64,NN=4096,H=1024)
+ decoder, on 8 NeuronCores.

Strategy: gate-dimension sharding (each core owns a 128-wide hidden slice =
384 gate rows per layer), software-pipelined layers, one fused AllGather of
both layers' hidden-state chunks per pipeline slot.  All GEMMs keep the
stationary operand tiny (h.T chunks [128,16]) and stream weights as the
moving operand (fp32, N=384).  The layer-0 input projection is interleaved
into the PE stream so it hides under the early slots' AllGather windows.
"""
import numpy as np

import concourse.bacc as bacc
import concourse.tile as tile
import concourse.mybir as mybir
from concourse import bass_utils

B, T, NN, H = 16, 64, 4096, 1024
G3 = 3 * H                      # 3072 gate rows per layer
NC = 8                          # cores
HC = H // NC                    # 128 hidden per core
GC = 3 * HC                     # 384 gate rows per core
DC = NN // NC                   # 512 decoder rows per core
KH = H // 128                   # 8 K-chunks over hidden
KX = NN // 128                  # 32 K-chunks over input features
MT = (B * T) // 128             # 8 token blocks
fp32 = mybir.dt.float32

_CACHE = {}


def _build(debug=False):
    nc = bacc.Bacc("TRN2", target_bir_lowering=False, debug=False,
                   enable_asserts=False, num_devices=NC)
    d = {}
    # ---- DRAM I/O ----
    d["xT"] = nc.dram_tensor("xT", [KX, MT, 128, 128], fp32, kind="ExternalInput").ap()
    d["wih0"] = nc.dram_tensor("wih0", [128, KX * GC], fp32, kind="ExternalInput").ap()
    d["whh0"] = nc.dram_tensor("whh0", [128, KH * GC], fp32, kind="ExternalInput").ap()
    d["wih1"] = nc.dram_tensor("wih1", [128, KH * GC], fp32, kind="ExternalInput").ap()
    d["whh1"] = nc.dram_tensor("whh1", [128, KH * GC], fp32, kind="ExternalInput").ap()
    d["decw"] = nc.dram_tensor("decw", [128, KH * DC], fp32, kind="ExternalInput").ap()
    d["bih0"] = nc.dram_tensor("bih0", [1, GC], fp32, kind="ExternalInput").ap()
    d["bhh0"] = nc.dram_tensor("bhh0", [1, GC], fp32, kind="ExternalInput").ap()
    d["bih1"] = nc.dram_tensor("bih1", [1, GC], fp32, kind="ExternalInput").ap()
    d["bhh1"] = nc.dram_tensor("bhh1", [1, GC], fp32, kind="ExternalInput").ap()
    d["decb"] = nc.dram_tensor("decb", [1, DC], fp32, kind="ExternalInput").ap()
    d["ones"] = nc.dram_tensor("ones", [1, 128], fp32, kind="ExternalInput").ap()
    d["eye16"] = nc.dram_tensor("eye16", [16, 16], fp32, kind="ExternalInput").ap()
    out_d = nc.dram_tensor("out", [B, DC], fp32, kind="ExternalOutput").ap()
    if debug:
        dbg_gx0 = nc.dram_tensor("dbg_gx0", [T, B, GC], fp32, kind="ExternalOutput").ap()
        dbg_ag0 = nc.dram_tensor("dbg_ag0", [2 * 128 * NC, 16], fp32, kind="ExternalOutput").ap()
        dbg_ag1 = nc.dram_tensor("dbg_ag1", [2 * 128 * NC, 16], fp32, kind="ExternalOutput").ap()

    S = mybir.ActivationFunctionType.Sigmoid
    TA = mybir.ActivationFunctionType.Tanh

    with tile.TileContext(nc) as tc:
        with tc.tile_pool(name="wsb", bufs=1) as wsb, \
             tc.tile_pool(name="xp", bufs=6) as xp, \
             tc.tile_pool(name="gxp", bufs=3) as gxp, \
             tc.tile_pool(name="hp", bufs=2) as hp, \
             tc.tile_pool(name="gp", bufs=2) as gp, \
             tc.tile_pool(name="ps", bufs=1, space="PSUM") as ps, \
             tc.tile_pool(name="dram", bufs=1, space="DRAM") as drp, \
             tc.tile_pool(name="agd", bufs=3, space="DRAM") as agd:

            # ---- persistent SBUF loads ----
            wih0 = wsb.tile([128, KX * GC], fp32, tag="wih0")
            whh0 = wsb.tile([128, KH * GC], fp32, tag="whh0")
            wih1 = wsb.tile([128, KH * GC], fp32, tag="wih1")
            whh1 = wsb.tile([128, KH * GC], fp32, tag="whh1")
            decw = wsb.tile([128, KH * DC], fp32, tag="decw")
            nc.sync.dma_start(out=wih0[:], in_=d["wih0"])
            nc.sync.dma_start(out=whh0[:], in_=d["whh0"])
            nc.sync.dma_start(out=wih1[:], in_=d["wih1"])
            nc.sync.dma_start(out=whh1[:], in_=d["whh1"])
            nc.sync.dma_start(out=decw[:], in_=d["decw"])
            bias = {}
            for nm in ("bih0", "bhh0", "bih1", "bhh1"):
                bias[nm] = wsb.tile([1, GC], fp32, tag=nm, name=nm)
                nc.sync.dma_start(out=bias[nm][:], in_=d[nm])
            decb = wsb.tile([1, DC], fp32, tag="decb")
            nc.sync.dma_start(out=decb[:], in_=d["decb"])
            ones = wsb.tile([1, 128], fp32, tag="ones")
            nc.sync.dma_start(out=ones[:], in_=d["ones"])
            eye16 = wsb.tile([16, 16], fp32, tag="eye16")
            nc.sync.dma_start(out=eye16[:], in_=d["eye16"])

            zero16 = wsb.tile([16, HC], fp32, tag="zero16")
            nc.vector.memset(zero16[:], 0.0)

            # gx0 staging in DRAM: [T, 16, GC]
            gx0_dram = drp.tile([T, B, GC], fp32, tag="gx0d")

            # ---- layer-0 projection, emitted in chunks ----
            def emit_proj_chunk(m):
                pp = ps.tile([128, GC], fp32, tag=f"proj{m % 2}")
                for k in range(KX):
                    xt = xp.tile([128, 128], fp32, tag="xt")
                    nc.sync.dma_start(out=xt[:], in_=d["xT"][k, m])
                    nc.tensor.matmul(pp[:], xt[:], wih0[:, k * GC:(k + 1) * GC],
                                     start=(k == 0), stop=False)
                nc.tensor.matmul(pp[:], ones[0:1, 0:128], bias["bih0"][:],
                                 start=False, stop=True)
                psb = xp.tile([128, GC], fp32, tag="projsb")
                nc.vector.tensor_copy(psb[:], pp[:])
                nc.sync.dma_start(
                    out=gx0_dram[m * 8:(m + 1) * 8].rearrange("t b g -> (t b) g"),
                    in_=psb[:])
                if debug:
                    nc.sync.dma_start(
                        out=dbg_gx0[m * 8:(m + 1) * 8].rearrange("t b g -> (t b) g"),
                        in_=psb[:])

            # per-slot state
            h1T_prev = None      # [128, 128] SBUF: h1(s-1).T as 8 chunks of 16
            h2T_prev = None
            h1_old = zero16      # [16,128] my slice of h1(s-1)
            h2_old = zero16

            def gates(gh_ps, gx_ap, h_old, tagpfx):
                """gh_ps: PSUM [16,GC] (includes b_hh); gx_ap: [16,GC] (includes
                b_ih). Returns h_new [16,HC] SBUF tile."""
                pre = gp.tile([16, 2 * HC], fp32, tag=f"{tagpfx}pre")
                rz = gp.tile([16, 2 * HC], fp32, tag=f"{tagpfx}rz")
                c0 = gp.tile([16, HC], fp32, tag=f"{tagpfx}c0")
                d0 = gp.tile([16, HC], fp32, tag=f"{tagpfx}d0")
                n0 = gp.tile([16, HC], fp32, tag=f"{tagpfx}n0")
                e0 = gp.tile([16, HC], fp32, tag=f"{tagpfx}e0")
                f0 = gp.tile([16, HC], fp32, tag=f"{tagpfx}f0")
                hn = gp.tile([16, HC], fp32, tag=f"{tagpfx}hn")
                # pre_r,pre_z = gx[0:256] + gh[0:256]
                nc.vector.tensor_add(pre[:], gx_ap[:, 0:2 * HC], gh_ps[:, 0:2 * HC])
                nc.scalar.activation(rz[:], pre[:], S)
                # c = r * gh_n ; d = gx_n + c ; n = tanh(d)
                nc.vector.tensor_mul(c0[:], rz[:, 0:HC], gh_ps[:, 2 * HC:3 * HC])
                nc.vector.tensor_add(d0[:], gx_ap[:, 2 * HC:3 * HC], c0[:])
                nc.scalar.activation(n0[:], d0[:], TA)
                # h_new = n + z*(h_old - n)
                nc.vector.tensor_sub(e0[:], h_old[:], n0[:])
                nc.vector.tensor_mul(f0[:], rz[:, HC:2 * HC], e0[:])
                hnew = gp.tile([16, HC], fp32, tag=f"{tagpfx}hnew")
                nc.vector.tensor_add(hnew[:], n0[:], f0[:])
                return hnew

            proj_sched = {0: [0, 1]}
            for j in range(6):
                proj_sched.setdefault(1 + j * 4, []).append(2 + j)

            for s in range(T + 1):
                for m in proj_sched.get(s, []):
                    emit_proj_chunk(m)

                agin = agd.tile([2 * 128, 16], fp32, tag="agin")
                # ---------- layer 0, step s ----------
                if s < T:
                    gh0 = ps.tile([16, GC], fp32, tag="gh0")
                    if s == 0:
                        nc.tensor.matmul(gh0[:], ones[0:1, 0:16], bias["bhh0"][:],
                                         start=True, stop=True)
                    else:
                        for k in range(KH):
                            nc.tensor.matmul(gh0[:], h1T_prev[:, k * 16:(k + 1) * 16],
                                             whh0[:, k * GC:(k + 1) * GC],
                                             start=(k == 0), stop=False)
                        nc.tensor.matmul(gh0[:], ones[0:1, 0:16], bias["bhh0"][:],
                                         start=False, stop=True)
                    gx0 = gxp.tile([16, GC], fp32, tag="gx0")
                    nc.sync.dma_start(out=gx0[:], in_=gx0_dram[s].opt())
                    h1_new = gates(gh0, gx0, h1_old, "l0")
                    t0 = ps.tile([128, 16], fp32, tag="t0")
                    nc.tensor.transpose(t0[:], h1_new[:], eye16[:])
                    t0s = gp.tile([128, 16], fp32, tag="t0s")
                    nc.vector.tensor_copy(t0s[:], t0[:])
                    nc.sync.dma_start(out=agin[0:128, :], in_=t0s[:])
                    h1_old = h1_new

                # ---------- layer 1, step s-1 ----------
                if s >= 1:
                    gx1 = ps.tile([16, GC], fp32, tag="gx1")
                    for k in range(KH):
                        nc.tensor.matmul(gx1[:], h1T_prev[:, k * 16:(k + 1) * 16],
                                         wih1[:, k * GC:(k + 1) * GC],
                                         start=(k == 0), stop=False)
                    nc.tensor.matmul(gx1[:], ones[0:1, 0:16], bias["bih1"][:],
                                     start=False, stop=True)
                    gh1 = ps.tile([16, GC], fp32, tag="gh1")
                    if s == 1:
                        nc.tensor.matmul(gh1[:], ones[0:1, 0:16], bias["bhh1"][:],
                                         start=True, stop=True)
                    else:
                        for k in range(KH):
                            nc.tensor.matmul(gh1[:], h2T_prev[:, k * 16:(k + 1) * 16],
                                             whh1[:, k * GC:(k + 1) * GC],
                                             start=(k == 0), stop=False)
                        nc.tensor.matmul(gh1[:], ones[0:1, 0:16], bias["bhh1"][:],
                                         start=False, stop=True)
                    gx1s = gxp.tile([16, GC], fp32, tag="gx1s")
                    nc.vector.tensor_copy(gx1s[:], gx1[:])
                    h2_new = gates(gh1, gx1s[:], h2_old, "l1")
                    t1 = ps.tile([128, 16], fp32, tag="t1")
                    nc.tensor.transpose(t1[:], h2_new[:], eye16[:])
                    t1s = gp.tile([128, 16], fp32, tag="t1s")
                    nc.vector.tensor_copy(t1s[:], t1[:])
                    nc.sync.dma_start(out=agin[128:256, :], in_=t1s[:])
                    h2_old = h2_new

                # ---------- fused AllGather ----------
                agout = agd.tile([2 * 128 * NC, 16], fp32, tag="agout")
                nc.gpsimd.collective_compute(
                    "AllGather", mybir.AluOpType.bypass,
                    replica_groups=[list(range(NC))],
                    ins=[agin.opt()], outs=[agout.opt()])
                if debug and s == 0:
                    nc.sync.dma_start(out=dbg_ag0, in_=agout[:])
                if debug and s == 1:
                    nc.sync.dma_start(out=dbg_ag1, in_=agout[:])
                if s < T:
                    h1T = hp.tile([128, 128], fp32, tag="h1T")
                    nc.sync.dma_start(
                        out=h1T[:].rearrange("p (r b) -> p r b", r=8),
                        in_=agout[:].rearrange("(r u p) b -> p r u b", u=2, p=128)[:, :, 0, :])
                    h1T_prev = h1T
                if s >= 1:
                    h2T = hp.tile([128, 128], fp32, tag="h2T")
                    nc.sync.dma_start(
                        out=h2T[:].rearrange("p (r b) -> p r b", r=8),
                        in_=agout[:].rearrange("(r u p) b -> p r u b", u=2, p=128)[:, :, 1, :])
                    h2T_prev = h2T

            # ---------- decoder: out = h2(T-1) @ dec_w_c.T + dec_b_c ----------
            pd = ps.tile([16, DC], fp32, tag="dec")
            for k in range(KH):
                nc.tensor.matmul(pd[:], h2T_prev[:, k * 16:(k + 1) * 16],
                                 decw[:, k * DC:(k + 1) * DC],
                                 start=(k == 0), stop=False)
            nc.tensor.matmul(pd[:], ones[0:1, 0:16], decb[:], start=False, stop=True)
            od = gp.tile([16, DC], fp32, tag="od")
            nc.vector.tensor_copy(od[:], pd[:])
            nc.sync.dma_start(out=out_d, in_=od[:])


    nc.compile()
    return nc


def _gate_rows(c):
    """Row indices (into 3H) owned by core c: r, z, n sections of its slice."""
    sl = np.arange(c * HC, (c + 1) * HC)
    return np.concatenate([sl, H + sl, 2 * H + sl])


def kernel(*a, **kw):
    out, _ = _run(False, *a, **kw)
    return out


def kernel_dbg(*a, **kw):
    return _run(True, *a, **kw)


def _run(debug, x, w_ih_l0, w_hh_l0, b_ih_l0, b_hh_l0,
         w_ih_l1, w_hh_l1, b_ih_l1, b_hh_l1, dec_w, dec_b):
    key = ("dbg" if debug else "nc")
    if key not in _CACHE:
        _CACHE[key] = _build(debug)
    nc = _CACHE[key]

    x = np.asarray(x, np.float32)
    # xT tiled: [KX, MT, 128, 128]; token index = t*16+b
    xT = np.ascontiguousarray(x.transpose(2, 1, 0).reshape(NN, T * B))
    xT_t = np.ascontiguousarray(
        xT.reshape(KX, 128, MT, 128).transpose(0, 2, 1, 3))

    def pack_kT(w_rows, kchunks, ncols):
        """w_rows [ncols_rows, K]: -> packed [128, kchunks*ncols] where
        packed[p, k*ncols+g] = w_rows[g, k*128+p] (i.e. w_rows.T chunks)."""
        wT = np.ascontiguousarray(np.asarray(w_rows, np.float32).T)  # [K, ncols]
        return np.ascontiguousarray(
            wT.reshape(kchunks, 128, ncols).transpose(1, 0, 2).reshape(128, kchunks * ncols))

    ones = np.ones((1, 128), np.float32)
    eye16 = np.eye(16, dtype=np.float32)

    in_maps = []
    for c in range(NC):
        rows = _gate_rows(c)
        drows = slice(c * DC, (c + 1) * DC)
        m = {
            "xT": xT_t,
            "wih0": pack_kT(np.asarray(w_ih_l0)[rows], KX, GC),
            "whh0": pack_kT(np.asarray(w_hh_l0)[rows], KH, GC),
            "wih1": pack_kT(np.asarray(w_ih_l1)[rows], KH, GC),
            "whh1": pack_kT(np.asarray(w_hh_l1)[rows], KH, GC),
            "decw": pack_kT(np.asarray(dec_w)[drows], KH, DC),
            "bih0": np.asarray(b_ih_l0, np.float32)[rows][None, :],
            "bhh0": np.asarray(b_hh_l0, np.float32)[rows][None, :],
            "bih1": np.asarray(b_ih_l1, np.float32)[rows][None, :],
            "bhh1": np.asarray(b_hh_l1, np.float32)[rows][None, :],
            "decb": np.asarray(dec_b, np.float32)[drows][None, :],
            "ones": ones, "eye16": eye16,
        }
        in_maps.append(m)

    _CACHE["last_in_maps"] = in_maps
    res = bass_utils.run_bass_kernel_spmd(
        nc, in_maps, core_ids=list(range(NC)), trace=False)
    out = np.concatenate([res.results[c]["out"] for c in range(NC)], axis=1)
    return out, res


# revision 14
# speedup vs baseline: 14.0752x; 14.0752x over previous
"""Trainium2 Bass kernel for nn_BaselineGRU: 2-layer GRU (B=16,T=64,NN=4096,H=1024)
+ decoder, on 8 NeuronCores.

Strategy: gate-dimension sharding (each core owns a 128-wide hidden slice =
384 gate rows per layer), software-pipelined layers, one fused AllGather of
both layers' hidden-state chunks per pipeline slot.  All GEMMs keep the
stationary operand tiny (h.T chunks [128,16]) and stream weights as the
moving operand (fp32, N=384).  The layer-0 input projection is interleaved
into the PE stream so it hides under the early slots' AllGather windows.
"""
import numpy as np

import concourse.bacc as bacc
import concourse.tile as tile
import concourse.mybir as mybir
from concourse import bass_utils

B, T, NN, H = 16, 64, 4096, 1024
G3 = 3 * H                      # 3072 gate rows per layer
NC = 8                          # cores
HC = H // NC                    # 128 hidden per core
GC = 3 * HC                     # 384 gate rows per core
DC = NN // NC                   # 512 decoder rows per core
KH = H // 128                   # 8 K-chunks over hidden
KX = NN // 128                  # 32 K-chunks over input features
MT = (B * T) // 128             # 8 token blocks
fp32 = mybir.dt.float32

_CACHE = {}


def _build(debug=False, no_collective=False):
    nc = bacc.Bacc("TRN2", target_bir_lowering=False, debug=False,
                   enable_asserts=False, num_devices=NC)
    d = {}
    # ---- DRAM I/O ----
    d["xT"] = nc.dram_tensor("xT", [KX, MT, 128, 128], fp32, kind="ExternalInput").ap()
    d["wih0"] = nc.dram_tensor("wih0", [128, KX * GC], fp32, kind="ExternalInput").ap()
    d["whh0"] = nc.dram_tensor("whh0", [128, KH * GC], fp32, kind="ExternalInput").ap()
    d["wih1"] = nc.dram_tensor("wih1", [128, KH * GC], fp32, kind="ExternalInput").ap()
    d["whh1"] = nc.dram_tensor("whh1", [128, KH * GC], fp32, kind="ExternalInput").ap()
    d["decw"] = nc.dram_tensor("decw", [128, KH * DC], fp32, kind="ExternalInput").ap()
    d["bih0"] = nc.dram_tensor("bih0", [1, GC], fp32, kind="ExternalInput").ap()
    d["bhh0"] = nc.dram_tensor("bhh0", [1, GC], fp32, kind="ExternalInput").ap()
    d["bih1"] = nc.dram_tensor("bih1", [1, GC], fp32, kind="ExternalInput").ap()
    d["bhh1"] = nc.dram_tensor("bhh1", [1, GC], fp32, kind="ExternalInput").ap()
    d["decb"] = nc.dram_tensor("decb", [1, DC], fp32, kind="ExternalInput").ap()
    d["ones"] = nc.dram_tensor("ones", [1, 128], fp32, kind="ExternalInput").ap()
    d["eye16"] = nc.dram_tensor("eye16", [16, 16], fp32, kind="ExternalInput").ap()
    out_d = nc.dram_tensor("out", [B, DC], fp32, kind="ExternalOutput").ap()
    if debug:
        dbg_gx0 = nc.dram_tensor("dbg_gx0", [T, B, GC], fp32, kind="ExternalOutput").ap()
        dbg_ag0 = nc.dram_tensor("dbg_ag0", [2 * 128 * NC, 16], fp32, kind="ExternalOutput").ap()
        dbg_ag1 = nc.dram_tensor("dbg_ag1", [2 * 128 * NC, 16], fp32, kind="ExternalOutput").ap()

    S = mybir.ActivationFunctionType.Sigmoid
    TA = mybir.ActivationFunctionType.Tanh

    with tile.TileContext(nc) as tc:
        with tc.tile_pool(name="wsb", bufs=1) as wsb, \
             tc.tile_pool(name="xp", bufs=6) as xp, \
             tc.tile_pool(name="gxp", bufs=3) as gxp, \
             tc.tile_pool(name="hp", bufs=2) as hp, \
             tc.tile_pool(name="gp", bufs=2) as gp, \
             tc.tile_pool(name="ps", bufs=1, space="PSUM") as ps, \
             tc.tile_pool(name="dram", bufs=1, space="DRAM") as drp, \
             tc.tile_pool(name="agd", bufs=16, space="DRAM") as agd:

            # ---- persistent SBUF loads ----
            wih0 = wsb.tile([128, KX * GC], fp32, tag="wih0")
            whh0 = wsb.tile([128, KH * GC], fp32, tag="whh0")
            wih1 = wsb.tile([128, KH * GC], fp32, tag="wih1")
            whh1 = wsb.tile([128, KH * GC], fp32, tag="whh1")
            decw = wsb.tile([128, KH * DC], fp32, tag="decw")
            nc.sync.dma_start(out=wih0[:], in_=d["wih0"])
            nc.sync.dma_start(out=whh0[:], in_=d["whh0"])
            nc.sync.dma_start(out=wih1[:], in_=d["wih1"])
            nc.sync.dma_start(out=whh1[:], in_=d["whh1"])
            nc.sync.dma_start(out=decw[:], in_=d["decw"])
            bias = {}
            for nm in ("bih0", "bhh0", "bih1", "bhh1"):
                bias[nm] = wsb.tile([1, GC], fp32, tag=nm, name=nm)
                nc.sync.dma_start(out=bias[nm][:], in_=d[nm])
            decb = wsb.tile([1, DC], fp32, tag="decb")
            nc.sync.dma_start(out=decb[:], in_=d["decb"])
            ones = wsb.tile([1, 128], fp32, tag="ones")
            nc.sync.dma_start(out=ones[:], in_=d["ones"])
            eye16 = wsb.tile([16, 16], fp32, tag="eye16")
            nc.sync.dma_start(out=eye16[:], in_=d["eye16"])

            zero16 = wsb.tile([16, HC], fp32, tag="zero16")
            nc.vector.memset(zero16[:], 0.0)

            # gx0 staging in DRAM: [T, 16, GC]
            gx0_dram = drp.tile([T, B, GC], fp32, tag="gx0d")

            # ---- layer-0 projection, emitted in chunks ----
            def emit_proj_chunk(m):
                pp = ps.tile([128, GC], fp32, tag=f"proj{m % 2}")
                for k in range(KX):
                    xt = xp.tile([128, 128], fp32, tag="xt")
                    nc.sync.dma_start(out=xt[:], in_=d["xT"][k, m])
                    nc.tensor.matmul(pp[:], xt[:], wih0[:, k * GC:(k + 1) * GC],
                                     start=(k == 0), stop=False)
                nc.tensor.matmul(pp[:], ones[0:1, 0:128], bias["bih0"][:],
                                 start=False, stop=True)
                psb = xp.tile([128, GC], fp32, tag="projsb")
                nc.vector.tensor_copy(psb[:], pp[:])
                nc.sync.dma_start(
                    out=gx0_dram[m * 8:(m + 1) * 8].rearrange("t b g -> (t b) g"),
                    in_=psb[:])
                if debug:
                    nc.sync.dma_start(
                        out=dbg_gx0[m * 8:(m + 1) * 8].rearrange("t b g -> (t b) g"),
                        in_=psb[:])

            # per-slot state
            h1T_prev = None      # [128, 128] SBUF: h1(s-1).T as 8 chunks of 16
            h2T_prev = None
            h1_old = zero16      # [16,128] my slice of h1(s-1)
            h2_old = zero16

            def gates(gh_ps, gx_ap, h_old, tagpfx):
                """gh_ps: PSUM [16,GC] (includes b_hh); gx_ap: [16,GC] (includes
                b_ih). Returns h_new [16,HC] SBUF tile."""
                pre = gp.tile([16, 2 * HC], fp32, tag=f"{tagpfx}pre")
                rz = gp.tile([16, 2 * HC], fp32, tag=f"{tagpfx}rz")
                c0 = gp.tile([16, HC], fp32, tag=f"{tagpfx}c0")
                d0 = gp.tile([16, HC], fp32, tag=f"{tagpfx}d0")
                n0 = gp.tile([16, HC], fp32, tag=f"{tagpfx}n0")
                e0 = gp.tile([16, HC], fp32, tag=f"{tagpfx}e0")
                f0 = gp.tile([16, HC], fp32, tag=f"{tagpfx}f0")
                hn = gp.tile([16, HC], fp32, tag=f"{tagpfx}hn")
                # pre_r,pre_z = gx[0:256] + gh[0:256]
                nc.vector.tensor_add(pre[:], gx_ap[:, 0:2 * HC], gh_ps[:, 0:2 * HC])
                nc.scalar.activation(rz[:], pre[:], S)
                # c = r * gh_n ; d = gx_n + c ; n = tanh(d)
                nc.vector.tensor_mul(c0[:], rz[:, 0:HC], gh_ps[:, 2 * HC:3 * HC])
                nc.vector.tensor_add(d0[:], gx_ap[:, 2 * HC:3 * HC], c0[:])
                nc.scalar.activation(n0[:], d0[:], TA)
                # h_new = n + z*(h_old - n)
                nc.vector.tensor_sub(e0[:], h_old[:], n0[:])
                nc.vector.tensor_mul(f0[:], rz[:, HC:2 * HC], e0[:])
                hnew = gp.tile([16, HC], fp32, tag=f"{tagpfx}hnew")
                nc.vector.tensor_add(hnew[:], n0[:], f0[:])
                return hnew

            proj_sched = {0: [0, 1]}
            for j in range(6):
                proj_sched.setdefault(1 + j * 4, []).append(2 + j)

            for s in range(T + 1):
                for m in proj_sched.get(s, []):
                    emit_proj_chunk(m)

                agin = agd.tile([2 * 128, 16], fp32, tag="agin")
                # ---------- layer 0, step s ----------
                if s < T:
                    gh0 = ps.tile([16, GC], fp32, tag="gh0")
                    if s == 0:
                        nc.tensor.matmul(gh0[:], ones[0:1, 0:16], bias["bhh0"][:],
                                         start=True, stop=True)
                    else:
                        for k in range(KH):
                            nc.tensor.matmul(gh0[:], h1T_prev[:, k * 16:(k + 1) * 16],
                                             whh0[:, k * GC:(k + 1) * GC],
                                             start=(k == 0), stop=False)
                        nc.tensor.matmul(gh0[:], ones[0:1, 0:16], bias["bhh0"][:],
                                         start=False, stop=True)
                    gx0 = gxp.tile([16, GC], fp32, tag="gx0")
                    nc.sync.dma_start(out=gx0[:], in_=gx0_dram[s].opt())
                    h1_new = gates(gh0, gx0, h1_old, "l0")
                    t0 = ps.tile([128, 16], fp32, tag="t0")
                    nc.tensor.transpose(t0[:], h1_new[:], eye16[:])
                    t0s = gp.tile([128, 16], fp32, tag="t0s")
                    nc.vector.tensor_copy(t0s[:], t0[:])
                    nc.sync.dma_start(out=agin[0:128, :], in_=t0s[:])
                    h1_old = h1_new

                # ---------- layer 1, step s-1 ----------
                if s >= 1:
                    gx1 = ps.tile([16, GC], fp32, tag="gx1")
                    for k in range(KH):
                        nc.tensor.matmul(gx1[:], h1T_prev[:, k * 16:(k + 1) * 16],
                                         wih1[:, k * GC:(k + 1) * GC],
                                         start=(k == 0), stop=False)
                    nc.tensor.matmul(gx1[:], ones[0:1, 0:16], bias["bih1"][:],
                                     start=False, stop=True)
                    gh1 = ps.tile([16, GC], fp32, tag="gh1")
                    if s == 1:
                        nc.tensor.matmul(gh1[:], ones[0:1, 0:16], bias["bhh1"][:],
                                         start=True, stop=True)
                    else:
                        for k in range(KH):
                            nc.tensor.matmul(gh1[:], h2T_prev[:, k * 16:(k + 1) * 16],
                                             whh1[:, k * GC:(k + 1) * GC],
                                             start=(k == 0), stop=False)
                        nc.tensor.matmul(gh1[:], ones[0:1, 0:16], bias["bhh1"][:],
                                         start=False, stop=True)
                    gx1s = gxp.tile([16, GC], fp32, tag="gx1s")
                    nc.vector.tensor_copy(gx1s[:], gx1[:])
                    h2_new = gates(gh1, gx1s[:], h2_old, "l1")
                    t1 = ps.tile([128, 16], fp32, tag="t1")
                    nc.tensor.transpose(t1[:], h2_new[:], eye16[:])
                    t1s = gp.tile([128, 16], fp32, tag="t1s")
                    nc.vector.tensor_copy(t1s[:], t1[:])
                    nc.sync.dma_start(out=agin[128:256, :], in_=t1s[:])
                    h2_old = h2_new

                # ---------- fused AllGather ----------
                agout = agd.tile([2 * 128 * NC, 16], fp32, tag="agout")
                if no_collective:
                    for r in range(NC):
                        nc.sync.dma_start(out=agout[256 * r:256 * (r + 1), :], in_=agin[:])
                else:
                    nc.gpsimd.collective_compute(
                        "AllGather", mybir.AluOpType.bypass,
                        replica_groups=[list(range(NC))],
                        ins=[agin.opt()], outs=[agout.opt()])
                if debug and s == 0:
                    nc.sync.dma_start(out=dbg_ag0, in_=agout[:])
                if debug and s == 1:
                    nc.sync.dma_start(out=dbg_ag1, in_=agout[:])
                if s < T:
                    h1T = hp.tile([128, 128], fp32, tag="h1T")
                    nc.sync.dma_start(
                        out=h1T[:].rearrange("p (r b) -> p r b", r=8),
                        in_=agout[:].rearrange("(r u p) b -> p r u b", u=2, p=128)[:, :, 0, :])
                    h1T_prev = h1T
                if s >= 1:
                    h2T = hp.tile([128, 128], fp32, tag="h2T")
                    nc.sync.dma_start(
                        out=h2T[:].rearrange("p (r b) -> p r b", r=8),
                        in_=agout[:].rearrange("(r u p) b -> p r u b", u=2, p=128)[:, :, 1, :])
                    h2T_prev = h2T

            # ---------- decoder: out = h2(T-1) @ dec_w_c.T + dec_b_c ----------
            pd = ps.tile([16, DC], fp32, tag="dec")
            for k in range(KH):
                nc.tensor.matmul(pd[:], h2T_prev[:, k * 16:(k + 1) * 16],
                                 decw[:, k * DC:(k + 1) * DC],
                                 start=(k == 0), stop=False)
            nc.tensor.matmul(pd[:], ones[0:1, 0:16], decb[:], start=False, stop=True)
            od = gp.tile([16, DC], fp32, tag="od")
            nc.vector.tensor_copy(od[:], pd[:])
            nc.sync.dma_start(out=out_d, in_=od[:])


    nc.compile()
    return nc


def _gate_rows(c):
    """Row indices (into 3H) owned by core c: r, z, n sections of its slice."""
    sl = np.arange(c * HC, (c + 1) * HC)
    return np.concatenate([sl, H + sl, 2 * H + sl])


def kernel(*a, **kw):
    out, _ = _run(False, *a, **kw)
    return out


def kernel_dbg(*a, **kw):
    return _run(True, *a, **kw)


def _run(debug, x, w_ih_l0, w_hh_l0, b_ih_l0, b_hh_l0,
         w_ih_l1, w_hh_l1, b_ih_l1, b_hh_l1, dec_w, dec_b):
    key = ("dbg" if debug else "nc")
    if key not in _CACHE:
        _CACHE[key] = _build(debug)
    nc = _CACHE[key]

    x = np.asarray(x, np.float32)
    # xT tiled: [KX, MT, 128, 128]; token index = t*16+b
    xT = np.ascontiguousarray(x.transpose(2, 1, 0).reshape(NN, T * B))
    xT_t = np.ascontiguousarray(
        xT.reshape(KX, 128, MT, 128).transpose(0, 2, 1, 3))

    def pack_kT(w_rows, kchunks, ncols):
        """w_rows [ncols_rows, K]: -> packed [128, kchunks*ncols] where
        packed[p, k*ncols+g] = w_rows[g, k*128+p] (i.e. w_rows.T chunks)."""
        wT = np.ascontiguousarray(np.asarray(w_rows, np.float32).T)  # [K, ncols]
        return np.ascontiguousarray(
            wT.reshape(kchunks, 128, ncols).transpose(1, 0, 2).reshape(128, kchunks * ncols))

    ones = np.ones((1, 128), np.float32)
    eye16 = np.eye(16, dtype=np.float32)

    in_maps = []
    for c in range(NC):
        rows = _gate_rows(c)
        drows = slice(c * DC, (c + 1) * DC)
        m = {
            "xT": xT_t,
            "wih0": pack_kT(np.asarray(w_ih_l0)[rows], KX, GC),
            "whh0": pack_kT(np.asarray(w_hh_l0)[rows], KH, GC),
            "wih1": pack_kT(np.asarray(w_ih_l1)[rows], KH, GC),
            "whh1": pack_kT(np.asarray(w_hh_l1)[rows], KH, GC),
            "decw": pack_kT(np.asarray(dec_w)[drows], KH, DC),
            "bih0": np.asarray(b_ih_l0, np.float32)[rows][None, :],
            "bhh0": np.asarray(b_hh_l0, np.float32)[rows][None, :],
            "bih1": np.asarray(b_ih_l1, np.float32)[rows][None, :],
            "bhh1": np.asarray(b_hh_l1, np.float32)[rows][None, :],
            "decb": np.asarray(dec_b, np.float32)[drows][None, :],
            "ones": ones, "eye16": eye16,
        }
        in_maps.append(m)

    _CACHE["last_in_maps"] = in_maps
    res = bass_utils.run_bass_kernel_spmd(
        nc, in_maps, core_ids=list(range(NC)), trace=False)
    out = np.concatenate([res.results[c]["out"] for c in range(NC)], axis=1)
    return out, res
